# revision 25
# baseline (speedup 1.0000x reference)
"""GPT transformer (L=4, T=2048, E=1024, H=16, FF=4096, V=32000) on 8 trn2 cores.

Sequence-sharded data parallel: core c owns tokens [c*256, (c+1)*256).
Weights replicated (fp16, streamed from HBM). Per layer one merged K+V
AllGather (fp16, ~1 MB per rank) gives every core the full sequence.

All device compute in fp16 (same PE rate as bf16, 8x less rounding noise);
fp32 residual stream; fp32 LayerNorm/softmax scalar paths (rsqrt via
Ln+Exp+Newton to stay inside the exp activation-table set). Device fp->fp16
cast bias is compensated by scaling host-prepped weights by 1/(1-2^-11) per
cast. Scores are computed per head-pair (row-tiled PE concurrency), exp is
batched [128,1024] across 4 key-tiles, softmax denominators ride a ones
column in V, reciprocals via reciprocal_approx_accurate. Logits leave in
fp16 (converted to fp32 on host).
"""
import sys

sys.path.insert(0, "/opt/trn_rl_repo")

import numpy as np

import concourse.tile as tile
from concourse import bacc, mybir
from concourse.bass_utils import run_bass_kernel_spmd

DT = mybir.dt.float16
F32 = mybir.dt.float32
AF = mybir.ActivationFunctionType
ALU = mybir.AluOpType

NC = 8
T, E, H, DH, L, FF, V = 2048, 1024, 16, 64, 4, 4096, 32000
TL = T // NC          # 256 tokens per core
ET = E // 128         # 8 feature tiles
FT = FF // 128        # 32 ff tiles
VT = V // 128         # 250 vocab tiles
VB = VT // 2          # 125 paired head blocks
KT = T // 128         # 16 key tiles
KVW = 2 * ET * 128 + 2 * H * (DH + 1)   # 2048 + 2080 = 4128
EPS = 1e-5
SCALE = float(1.0 / np.sqrt(DH))
C1 = 1.0    # device fp->fp16 casts measured round-to-nearest; no bias compensation

_CACHE = {}
DEBUG = False


def _build():
    nc = bacc.Bacc("TRN2", target_bir_lowering=False, debug=False,
                   enable_asserts=True, num_devices=NC)

    x0_in = nc.dram_tensor("x0", [128, ET, TL], F32, kind="ExternalInput").ap()
    mask_in = nc.dram_tensor("mask", [128, KT, TL], DT, kind="ExternalInput").ap()
    wq_in = [nc.dram_tensor(f"wq{l}", [ET, 128, E], DT, kind="ExternalInput").ap() for l in range(L)]
    wk_in = [nc.dram_tensor(f"wk{l}", [ET, 128, E], DT, kind="ExternalInput").ap() for l in range(L)]
    wv_in = [nc.dram_tensor(f"wv{l}", [ET, 128, E], DT, kind="ExternalInput").ap() for l in range(L)]
    wo_in = [nc.dram_tensor(f"wo{l}", [ET, 128, E], DT, kind="ExternalInput").ap() for l in range(L)]
    w1_in = [nc.dram_tensor(f"w1{l}", [FT, 128, E], DT, kind="ExternalInput").ap() for l in range(L)]
    w2_in = [nc.dram_tensor(f"w2{l}", [ET, 128, FF], DT, kind="ExternalInput").ap() for l in range(L)]
    wh_in = nc.dram_tensor("wh", [VB, 128, 2 * E], DT, kind="ExternalInput").ap()
    out_l = nc.dram_tensor("logits", [VB, 128, 2, TL], DT, kind="ExternalOutput").ap()

    kv_in = [nc.dram_tensor(f"kv_in{l}", [128, KVW], DT) for l in range(L)]
    kv_out = [nc.dram_tensor(f"kv_out{l}", [NC, 128, KVW], DT, addr_space="Shared") for l in range(L)]

    dbg = {}
    if DEBUG:
        dbg["xh1"] = nc.dram_tensor("d_xh1", [128, ET, TL], DT, kind="ExternalOutput").ap()
        dbg["q"] = nc.dram_tensor("d_q", [128, ET, TL], DT, kind="ExternalOutput").ap()
        dbg["kall"] = nc.dram_tensor("d_kall", [128, KT, ET, 128], DT, kind="ExternalOutput").ap()
        dbg["vall"] = nc.dram_tensor("d_vall", [128, KT, H, DH + 1], DT, kind="ExternalOutput").ap()
        dbg["obf"] = nc.dram_tensor("d_obf", [128, ET, TL], DT, kind="ExternalOutput").ap()
        dbg["xb"] = nc.dram_tensor("d_xb", [128, ET, TL], F32, kind="ExternalOutput").ap()
        dbg["xa1"] = nc.dram_tensor("d_xa1", [128, ET, TL], F32, kind="ExternalOutput").ap()

    with tile.TileContext(nc) as tc:
        import contextlib
        ctx = contextlib.ExitStack()
        singles = ctx.enter_context(tc.tile_pool(name="singles", bufs=1))
        xh_pool = ctx.enter_context(tc.tile_pool(name="xh", bufs=2))
        qk_pool = ctx.enter_context(tc.tile_pool(name="qk", bufs=1))
        pe_pool = ctx.enter_context(tc.tile_pool(name="pe", bufs=2))
        pm_pool = ctx.enter_context(tc.tile_pool(name="pm", bufs=2))
        st_pool = ctx.enter_context(tc.tile_pool(name="st", bufs=4))
        sm_pool = ctx.enter_context(tc.tile_pool(name="sm", bufs=8))
        rn_pool = ctx.enter_context(tc.tile_pool(name="rnp", bufs=2))
        f_pool = ctx.enter_context(tc.tile_pool(name="fp", bufs=1))
        wA_pool = ctx.enter_context(tc.tile_pool(name="wA", bufs=4))
        wV_pool = ctx.enter_context(tc.tile_pool(name="wV", bufs=3))
        wB_pool = ctx.enter_context(tc.tile_pool(name="wB", bufs=2))
        wH_pool = ctx.enter_context(tc.tile_pool(name="wH", bufs=3))
        lg_pool = ctx.enter_context(tc.tile_pool(name="lg", bufs=3))

        ps_big = ctx.enter_context(tc.tile_pool(name="ps_big", bufs=3, space="PSUM"))
        ps_o = ctx.enter_context(tc.tile_pool(name="ps_o", bufs=2, space="PSUM"))

        # persistent SBUF
        x_a = singles.tile([128, ET, TL], F32, tag="xa")       # residual stream
        x_b = singles.tile([128, ET, TL], F32, tag="xb")       # post-attention
        k_all = singles.tile([128, KT, ET, 128], DT, tag="kall")   # [p, ck, hp, t]
        v_all = singles.tile([128, KT, H, DH + 1], DT, tag="vall")  # [tk, ck, h, d|ones]
        o_bf = singles.tile([128, ET, TL], DT, tag="obf")
        mask_sb = singles.tile([128, KT, TL], DT, tag="mask")
        vsend = singles.tile([128, 2, H, DH + 1], DT, tag="vsend")
        k_loc = singles.tile([128, 2, ET, 128], DT, tag="kloc")    # [p, u, hp, t]
        ones_k = singles.tile([128, 1], DT, tag="onesk")
        ones_r = singles.tile([1, 128], DT, tag="onesr")

        nc.vector.memset(ones_k, 1.0)
        nc.vector.memset(ones_r, 1.0)
        nc.vector.memset(vsend[:, :, :, DH:DH + 1], 1.0)
        nc.sync.dma_start(out=mask_sb, in_=mask_in)
        nc.sync.dma_start(out=x_a, in_=x0_in)

        def layernorm(x_src, xh_dst):
            """xh_dst (fp16) = (x_src - mean) / sqrt(var + eps), per token.

            NB: the two accumulation groups MUST live in separate PSUM
            tiles (banks) — a group's start=True clears the whole bank's
            has_written bits, so interleaving two groups in one bank
            silently drops the other group's first partial sum.
            """
            s1_ps = ps_big.tile([1, TL], F32, tag="big")
            s2_ps = ps_big.tile([1, TL], F32, tag="big")
            for e in range(ET):
                xc = st_pool.tile([128, TL], DT, tag="stc")
                nc.vector.tensor_copy(out=xc, in_=x_src[:, e, :])
                nc.tensor.matmul(s1_ps, ones_k, xc,
                                 start=(e == 0), stop=(e == ET - 1))
                x2 = st_pool.tile([128, TL], DT, tag="stc")
                nc.scalar.activation(out=x2, in_=xc, func=AF.Square)
                nc.tensor.matmul(s2_ps, ones_k, x2,
                                 start=(e == 0), stop=(e == ET - 1))
            mu = sm_pool.tile([1, TL], F32, tag="sm")
            nc.vector.tensor_scalar_mul(mu, s1_ps, 1.0 / E)
            me2 = sm_pool.tile([1, TL], F32, tag="sm")
            nc.vector.tensor_scalar_mul(me2, s2_ps, 1.0 / E)
            mu2 = sm_pool.tile([1, TL], F32, tag="sm")
            nc.vector.tensor_mul(mu2, mu, mu)
            mu2e = sm_pool.tile([1, TL], F32, tag="sm")
            nc.vector.tensor_scalar(mu2e, mu2, EPS, None, op0=ALU.subtract)
            varp = sm_pool.tile([1, TL], F32, tag="sm")
            nc.vector.tensor_sub(varp, me2, mu2e)      # var + eps
            lnv = sm_pool.tile([1, TL], F32, tag="sm")
            nc.scalar.activation(out=lnv, in_=varp, func=AF.Ln)
            r0 = sm_pool.tile([1, TL], F32, tag="sm")
            nc.scalar.activation(out=r0, in_=lnv, func=AF.Exp, scale=-0.5)
            # one Newton step: r = r0 * (1.5 - 0.5 * varp * r0^2)
            t1 = sm_pool.tile([1, TL], F32, tag="sm")
            nc.vector.tensor_mul(t1, r0, r0)
            t2 = sm_pool.tile([1, TL], F32, tag="sm")
            nc.vector.tensor_mul(t2, t1, varp)
            coef = sm_pool.tile([1, TL], F32, tag="sm")
            nc.vector.tensor_scalar(coef, t2, -0.5, 1.5,
                                    op0=ALU.mult, op1=ALU.add)
            r = sm_pool.tile([1, TL], F32, tag="sm")
            nc.vector.tensor_mul(r, r0, coef)
            mur = sm_pool.tile([1, TL], F32, tag="sm")
            nc.vector.tensor_mul(mur, mu, r)
            rn = rn_pool.tile([1, 2 * TL], DT, tag="rn")
            nc.vector.tensor_scalar_mul(rn[:, 0:TL], r, C1)
            nc.vector.tensor_scalar_mul(rn[:, TL:2 * TL], mur, -C1)
            rb = ps_big.tile([128, 2 * TL], F32, tag="big")
            nc.tensor.matmul(rb, ones_r, rn, start=True, stop=True)
            for e in range(ET):
                t = st_pool.tile([128, TL], F32, tag="stf")
                nc.vector.tensor_mul(t, x_src[:, e, :], rb[:, 0:TL])
                nc.vector.tensor_add(xh_dst[:, e, :], t, rb[:, TL:2 * TL])

        def proj(w_dram, xh, n_m, n_k, wpool, wtag, consume):
            """consume(m, psum): psum = W @ xh feature-major out-tile m."""
            for m in range(n_m):
                wt = wpool.tile([128, n_k, 128], DT, tag=wtag)
                nc.sync.dma_start(
                    out=wt, in_=w_dram[m].rearrange("p (k j) -> p k j", j=128))
                ps = ps_big.tile([128, TL], F32, tag="big")
                for k in range(n_k):
                    nc.tensor.matmul(ps, wt[:, k, :], xh[:, k, :],
                                     start=(k == 0), stop=(k == n_k - 1))
                consume(m, ps)

        for l in range(L):
            xh1 = xh_pool.tile([128, ET, TL], DT, tag="xh")
            layernorm(x_a, xh1)

            # K projection (feature-major) -> k_loc [p, u, hp, t]
            proj(wk_in[l], xh1, ET, ET, wA_pool, "wA",
                 lambda m, ps: nc.vector.tensor_copy(
                     out=k_loc[:, :, m, :],
                     in_=ps.rearrange("p (u t) -> p u t", u=2)))

            # V projection (token-major, N=512) -> vsend
            for mt in range(2):
                ps0 = ps_big.tile([128, 512], F32, tag="big")
                ps1 = ps_big.tile([128, 512], F32, tag="big")
                pss = [ps0, ps1]
                for k in range(ET):
                    wvt = wV_pool.tile([128, E], DT, tag="wV")
                    nc.sync.dma_start(out=wvt, in_=wv_in[l][k])
                    for nh in range(2):
                        nc.tensor.matmul(
                            pss[nh], xh1[:, k, mt * 128:(mt + 1) * 128],
                            wvt[:, nh * 512:(nh + 1) * 512],
                            start=(k == 0), stop=(k == ET - 1))
                for nh in range(2):
                    nc.vector.tensor_copy(
                        out=vsend[:, mt, nh * 8:(nh + 1) * 8, 0:DH],
                        in_=pss[nh].rearrange("p (h d) -> p h d", d=DH))

            # merged K+V AllGather
            nc.sync.dma_start(out=kv_in[l].ap()[:, 0:2 * ET * 128], in_=k_loc)
            nc.sync.dma_start(out=kv_in[l].ap()[:, 2 * ET * 128:KVW], in_=vsend)
            nc.gpsimd.collective_compute(
                "AllGather", ALU.bypass, replica_groups=[list(range(NC))],
                ins=[kv_in[l].ap().opt()], outs=[kv_out[l].ap().opt()])

            # Q projection (overlaps the collective)
            q_bf = qk_pool.tile([128, ET, TL], DT, tag="qbf")
            proj(wq_in[l], xh1, ET, ET, wA_pool, "wA",
                 lambda m, ps: nc.vector.tensor_copy(out=q_bf[:, m, :], in_=ps))

            # load AllGathered K and V into SBUF (contiguous per rank)
            for r_ in range(NC):
                nc.sync.dma_start(
                    out=k_all[:, 2 * r_:2 * r_ + 2, :, :],
                    in_=kv_out[l].ap()[r_][:, 0:2 * ET * 128].rearrange(
                        "p (u h t) -> p u h t", u=2, h=ET))
                nc.sync.dma_start(
                    out=v_all[:, 2 * r_:2 * r_ + 2, :, :],
                    in_=kv_out[l].ap()[r_][:, 2 * ET * 128:KVW].rearrange(
                        "p (u h d) -> p u h d", u=2, h=H))

            # attention per head pair
            for hp in range(ET):
                o_e = ps_o.tile([DH + 1, TL], F32, tag="o")
                o_o = ps_o.tile([DH + 1, TL], F32, tag="o")
                for ckq in range(4):
                    se = ps_big.tile([128, 4 * TL], F32, tag="big")
                    so = ps_big.tile([128, 4 * TL], F32, tag="big")
                    for u in range(4):
                        ck = 4 * ckq + u
                        nc.tensor.matmul(se[:, u * TL:(u + 1) * TL],
                                         k_all[0:64, ck, hp, :],
                                         q_bf[0:64, hp, :],
                                         start=True, stop=True,
                                         skip_group_check=True)
                        nc.tensor.matmul(so[:, u * TL:(u + 1) * TL],
                                         k_all[64:128, ck, hp, :],
                                         q_bf[64:128, hp, :],
                                         start=True, stop=True,
                                         skip_group_check=True)
                    pe_e = pe_pool.tile([128, 4 * TL], DT, tag="pe")
                    nc.scalar.activation(out=pe_e, in_=se, func=AF.Exp, scale=SCALE)
                    pe_o = pe_pool.tile([128, 4 * TL], DT, tag="pe")
                    nc.scalar.activation(out=pe_o, in_=so, func=AF.Exp, scale=SCALE)
                    pm_e = pm_pool.tile([128, 4 * TL], DT, tag="pm")
                    nc.vector.tensor_mul(
                        pm_e, pe_e,
                        mask_sb[:, 4 * ckq:4 * ckq + 4, :].rearrange("p c t -> p (c t)"))
                    pm_o = pm_pool.tile([128, 4 * TL], DT, tag="pm")
                    nc.vector.tensor_mul(
                        pm_o, pe_o,
                        mask_sb[:, 4 * ckq:4 * ckq + 4, :].rearrange("p c t -> p (c t)"))
                    for u in range(4):
                        ck = 4 * ckq + u
                        nc.tensor.matmul(o_e, v_all[:, ck, 2 * hp, :],
                                         pm_e[:, u * TL:(u + 1) * TL],
                                         start=(ck == 0), stop=(ck == KT - 1))
                        nc.tensor.matmul(o_o, v_all[:, ck, 2 * hp + 1, :],
                                         pm_o[:, u * TL:(u + 1) * TL],
                                         start=(ck == 0), stop=(ck == KT - 1))
                # softmax epilogue for this head pair
                dd = sm_pool.tile([1, 2 * TL], DT, tag="dd")
                nc.vector.tensor_copy(out=dd[:, 0:TL], in_=o_e[DH:DH + 1, :])
                nc.vector.tensor_copy(out=dd[:, TL:2 * TL], in_=o_o[DH:DH + 1, :])
                rbp = ps_big.tile([128, TL], F32, tag="big")
                nc.tensor.matmul(rbp[0:64, :], ones_r[:, 0:64], dd[:, 0:TL],
                                 start=True, stop=True, skip_group_check=True)
                nc.tensor.matmul(rbp[64:128, :], ones_r[:, 0:64], dd[:, TL:2 * TL],
                                 start=True, stop=True, skip_group_check=True)
                rbf = st_pool.tile([128, TL], F32, tag="stf")
                scr2 = st_pool.tile([128, TL], F32, tag="stf")
                nc.vector.reciprocal_approx_accurate(rbf, rbp, scr2)
                nc.vector.tensor_mul(o_bf[0:64, hp, :], o_e[0:DH, :], rbf[0:64, :])
                nc.vector.tensor_mul(o_bf[64:128, hp, :], o_o[0:DH, :], rbf[64:128, :])

            if DEBUG and l == 0:
                nc.sync.dma_start(out=dbg["xh1"], in_=xh1)
                nc.sync.dma_start(out=dbg["q"], in_=q_bf)
                nc.sync.dma_start(out=dbg["kall"], in_=k_all)
                nc.sync.dma_start(out=dbg["vall"], in_=v_all)
                nc.sync.dma_start(out=dbg["obf"], in_=o_bf)

            # Wo projection + residual -> x_b
            proj(wo_in[l], o_bf, ET, ET, wA_pool, "wA",
                 lambda m, ps: nc.vector.tensor_add(x_b[:, m, :], ps, x_a[:, m, :]))
            if DEBUG and l == 0:
                nc.sync.dma_start(out=dbg["xb"], in_=x_b)

            # FFN
            xh2 = xh_pool.tile([128, ET, TL], DT, tag="xh")
            layernorm(x_b, xh2)
            f_bf = f_pool.tile([128, FT, TL], DT, tag="fbf")
            for mp in range(FT // 2):
                ps = ps_big.tile([128, 2 * TL], F32, tag="big")
                for mm in range(2):
                    wt = wA_pool.tile([128, ET, 128], DT, tag="wA")
                    nc.sync.dma_start(
                        out=wt,
                        in_=w1_in[l][2 * mp + mm].rearrange("p (k j) -> p k j", j=128))
                    for k in range(ET):
                        nc.tensor.matmul(ps[:, mm * TL:(mm + 1) * TL],
                                         wt[:, k, :], xh2[:, k, :],
                                         start=(k == 0), stop=(k == ET - 1),
                                         skip_group_check=True)
                nc.scalar.activation(
                    out=f_bf[:, 2 * mp:2 * mp + 2, :].rearrange("p c t -> p (c t)"),
                    in_=ps, func=AF.Gelu)
            proj(w2_in[l], f_bf, ET, FT, wB_pool, "wB",
                 lambda m, ps: nc.vector.tensor_add(x_a[:, m, :], ps, x_b[:, m, :]))
            if DEBUG and l == 0:
                nc.sync.dma_start(out=dbg["xa1"], in_=x_a)

        # final LN + head
        xhf = xh_pool.tile([128, ET, TL], DT, tag="xh")
        layernorm(x_a, xhf)

        for b in range(VB):
            wt = wH_pool.tile([128, 2, ET, 128], DT, tag="wH")
            nc.sync.dma_start(
                out=wt, in_=wh_in[b].rearrange("p (m k j) -> p m k j", m=2, j=128))
            lt = lg_pool.tile([128, 2, TL], DT, tag="lg")
            for mm in range(2):
                ps = ps_big.tile([128, TL], F32, tag="big")
                for k in range(ET):
                    nc.tensor.matmul(ps, wt[:, mm, k, :], xhf[:, k, :],
                                     start=(k == 0), stop=(k == ET - 1))
                if (2 * b + mm) % 2 == 0:
                    nc.scalar.copy(out=lt[:, mm, :], in_=ps)
                else:
                    nc.vector.tensor_copy(out=lt[:, mm, :], in_=ps)
            nc.sync.dma_start(out=out_l[b], in_=lt)

        ctx.close()
    nc.compile()
    return nc


def _prep_w_col(wT, n_k, n_m):
    """wT [K, M] -> [n_m, 128, K] with [m][p][(k j)] = wT[k*128+p, m*128+j]."""
    K, M = wT.shape
    a = wT.reshape(n_k, 128, n_m, 128).transpose(2, 1, 0, 3).reshape(n_m, 128, K)
    return np.ascontiguousarray(a).astype(np.float16)


def kernel(input_ids, tok_emb, pos_emb, Wq, bq, Wk, bk, Wv, bv, Wo, bo,
           ln1_g, ln1_b, W1, b1, W2, b2, ln2_g, ln2_b, lnf_g, lnf_b, Whead):
    input_ids = np.asarray(input_ids)
    ids = input_ids.reshape(-1).astype(np.int64)
    assert ids.shape[0] == T

    for b in (bq, bk, bv, bo, b1, b2, ln1_b, ln2_b, lnf_b):
        assert not np.any(np.asarray(b)), "nonzero biases not supported by this kernel"

    if "nc" not in _CACHE:
        _CACHE["nc"] = _build()
    nc = _CACHE["nc"]

    x0 = np.asarray(tok_emb)[ids] + np.asarray(pos_emb)   # [T, E] fp32
    x0 = x0.astype(np.float32)

    c2 = C1 * C1
    common = {}
    for l in range(L):
        g1 = np.asarray(ln1_g)[l][None, :]
        g2 = np.asarray(ln2_g)[l][None, :]
        common[f"wq{l}"] = _prep_w_col((np.asarray(Wq)[l] * g1 * c2).T, ET, ET)
        common[f"wk{l}"] = _prep_w_col((np.asarray(Wk)[l] * g1 * c2).T, ET, ET)
        wvT = (np.asarray(Wv)[l] * g1 * c2).T.astype(np.float16)   # [E_in, E_out]
        common[f"wv{l}"] = np.ascontiguousarray(wvT.reshape(ET, 128, E))
        common[f"wo{l}"] = _prep_w_col(np.asarray(Wo)[l].T * C1, ET, ET)
        common[f"w1{l}"] = _prep_w_col((np.asarray(W1)[l] * g2 * C1).T, ET, FT)
        common[f"w2{l}"] = _prep_w_col(np.asarray(W2)[l].T * C1, FT, ET)
    whT = (np.asarray(Whead) * np.asarray(lnf_g)[None, :] * c2).T  # [E, V]
    whB = whT.reshape(ET, 128, VB, 2, 128).transpose(2, 1, 3, 0, 4)
    common["wh"] = np.ascontiguousarray(whB.reshape(VB, 128, 2 * E)).astype(np.float16)


    k_global = np.arange(T).reshape(KT, 128)
    in_maps = []
    for c in range(NC):
        x0c = x0[c * TL:(c + 1) * TL].T.reshape(ET, 128, TL).transpose(1, 0, 2)
        q_global = c * TL + np.arange(TL)
        m = (k_global[:, :, None] <= q_global[None, None, :])
        mc = m.transpose(1, 0, 2).astype(np.float16)      # [128, KT, TL]
        im = dict(common)
        im["x0"] = np.ascontiguousarray(x0c)
        im["mask"] = np.ascontiguousarray(mc)
        in_maps.append(im)

    _CACHE["last_in_maps"] = in_maps
    res = run_bass_kernel_spmd(nc, in_maps, list(range(NC)))
    _CACHE["last_results"] = res

    logits = np.empty((1, T, V), dtype=np.float32)
    for c in range(NC):
        lg = res.results[c]["logits"]                     # [VB, 128, 2, TL] fp16
        logits[0, c * TL:(c + 1) * TL, :] = (
            lg.astype(np.float32).transpose(3, 0, 2, 1).reshape(TL, V))
    return logits
